# revision 5
# baseline (speedup 1.0000x reference)
"""EvoBinarizedLayer as one fp8 matmul per population member — v3.

Math (unchanged): count[p] = xb @ (w0 - w1) + colsum(w1) as a
[512,2048] @ [2048,2048] fp8 DoubleRow GEMM per core, fp32 PSUM, fp16
output + host bias/int32 upcast.

v4 schedule, tuned from the v2/v3 traces (45.25us / 47.47us):
- DMA supply: each HWDGE ring arms its first packet ~1.5-2.0us after
  the first dma_start (arming order sync/scalar is run-random), then
  sustains ~155 GB/s per ring. v3's fine-grained x|w interleave left
  ob1 with no prefetch margin and stalled 1.9us. v4 goes ob-major:
  sync ring carries x then ob1 then half of ob3; scalar carries ob0
  then ob2 then the other half of ob3 (2.5MB each). Every chunk then
  lands >=1us before its consumption time except the very first pair.
- HAM/pstate window needs ~3.2us of CONSECUTIVE gap-free PE
  streaming (hard-resets on ~100ns array-idle): warmups are N=384/512
  dummies (stream time covers NX issue), sized to end ~10.8us,
  slightly PAST the expected first-chunk semaphore (~10.7) — a late
  warmup end costs its overshoot once, an early one costs a window
  reset (~2us of half-rate matmuls). The final N=512 warmups absorb
  the real stream's first waits + LDWEIGHTS under their stream.
- Warm tile memset runs on GpSimd (v3's 1KiB/partition DVE memset
  took 911ns and delayed the first warmup).
- Tail: last o-quarter k-serial per batch tile; last batch tile is 4
  N=128 chains so the final drain is a ~290ns cast + 32KiB DMA.
"""

import numpy as np
import ml_dtypes

POP, BATCH, IN_INTS, OUT_F = 8, 512, 32, 2048
K = IN_INTS * 64          # 2048 contraction (bit) dim
KT = K // 128             # 16 k-tiles of 128
N_CORES = 8

_FP8 = ml_dtypes.float8_e4m3

_cached = {}


def _build_nc():
    import concourse.tile as tile
    from concourse import bacc, mybir

    dt = mybir.dt
    nc = bacc.Bacc(
        "TRN2", target_bir_lowering=False, debug=False, num_devices=N_CORES
    )
    xbt_d = nc.dram_tensor(
        "xbt", [4, 128, 4, BATCH], dt.float8e4, kind="ExternalInput"
    ).ap()
    wd_d = nc.dram_tensor(
        "wd", [4, 4, 128, 4, 512], dt.float8e4, kind="ExternalInput"
    ).ap()
    out_d = nc.dram_tensor(
        "out", [BATCH, OUT_F], dt.float16, kind="ExternalOutput"
    ).ap()

    DR = None  # set below once mybir import is live

    with tile.TileContext(nc) as tc:
        with (
            tc.tile_pool(name="xbt", bufs=1) as xbt_pool,
            tc.tile_pool(name="wd", bufs=1) as wd_pool,
            tc.tile_pool(name="outp", bufs=8) as out_pool,
            tc.tile_pool(name="psum", bufs=8, space="PSUM") as psum_pool,
        ):
            DR = mybir.MatmulPerfMode.DoubleRow
            xbt_sb = xbt_pool.tile([128, KT, BATCH], dt.float8e4)
            wd_sb = wd_pool.tile([128, 4, KT, 512], dt.float8e4)

            # Warmup: 9 N=512 dummies stream back-to-back for ~3.8us —
            # past the ~3.4us HAM warm-up window — so the PE is at full
            # clock BEFORE the real stream starts. The cold-phase
            # LDWEIGHTS double-buffer bubble (~200ns array idle after the
            # first real matmul) then lands post-open, where it no longer
            # re-throttles, instead of resetting the warm-up window.
            warm = xbt_pool.tile([128, 2, 512], dt.float8e4, tag="warm")
            nc.vector.memset(warm[:, 0:1, :], 0.0)
            nc.gpsimd.memset(warm[:, 1:2, :], 0.0)
            wps = psum_pool.tile([128, 512], dt.float32, tag="ps", name="ps_w")
            for _ in range(9):
                nc.tensor.matmul(
                    wps[:64, :], warm[:, :, :64], warm[:],
                    start=True, stop=True, perf_mode=DR,
                )

            # Input DMAs: [128,2,512] 128KiB k-pair chunks in consumption
            # order. sync: x kp0-7 then ob-even kps; scalar: ob0 kp0-7
            # then ob-odd kps. ob1/2/3 alternate rings by kp parity so
            # neither ring gates a whole o-quarter.
            def dma_w_kp(eng, ob, kp):
                kq, h = kp // 2, kp % 2
                eng.dma_start(
                    wd_sb[:, ob, 2 * kp : 2 * kp + 2, :],
                    wd_d[kq, ob][:, 2 * h : 2 * h + 2, :],
                )

            rr = [0]

            def next_ring():
                e = (nc.sync, nc.scalar)[rr[0] % 2]
                rr[0] += 1
                return e

            # ob0+ob1 run fused (below), so stream [x, w0, w1] triplets
            # per k-pair, strictly alternating rings; then ob2, then ob3.
            for kp in range(8):
                kq, h = kp // 2, kp % 2
                next_ring().dma_start(
                    xbt_sb[:, 2 * kp : 2 * kp + 2, :],
                    xbt_d[kq][:, 2 * h : 2 * h + 2, :],
                )
                dma_w_kp(next_ring(), 0, kp)
                dma_w_kp(next_ring(), 1, kp)
            for ob in range(2, 4):
                for kp in range(8):
                    dma_w_kp(next_ring(), ob, kp)

            out_rr = [0]

            def drain(ps, bt, ocol, width):
                ot = out_pool.tile(
                    [128, width], dt.float16, tag="ot",
                    name=f"ot_{bt}_{ocol}",
                )
                nc.vector.tensor_copy(ot[:], ps[:])
                eng = (nc.sync, nc.scalar)[out_rr[0] % 2]
                out_rr[0] += 1
                eng.dma_start(
                    out_d[128 * bt : 128 * (bt + 1), ocol : ocol + width],
                    ot[:],
                )

            # o-quarters 0+1 fused: k-pair outer over all 8 psum banks.
            # 8 matmuls per k-pair consume one [x, w0, w1] chunk triplet
            # (384KiB / 1.72us = 223 GB/s), leaving ~25% DMA-supply
            # margin where the per-quarter schedule rode at ~0 margin
            # and stalled whenever the rings armed slow.
            pss = [
                psum_pool.tile(
                    [128, 512], dt.float32, tag="ps", name=f"ps_{ob}_{bt}"
                )
                for ob in range(2)
                for bt in range(4)
            ]
            # bt-outer / ob-inner: consecutive matmuls share the same
            # stationary x-slice, halving LDWEIGHTS pressure (the
            # double-buffer bubble at stream start and quarter
            # boundaries comes from back-to-back stationary reloads).
            for k in range(KT // 2):
                for bt in range(4):
                    for ob in range(2):
                        nc.tensor.matmul(
                            pss[4 * ob + bt][:],
                            xbt_sb[:, 2 * k : 2 * k + 2, 128 * bt : 128 * (bt + 1)],
                            wd_sb[:, ob, 2 * k : 2 * k + 2, :],
                            start=(k == 0),
                            stop=(k == KT // 2 - 1),
                            perf_mode=DR,
                        )
            for ob in range(2):
                for bt in range(4):
                    drain(pss[4 * ob + bt], bt, 512 * ob, 512)

            # o-quarter 2: k-pair outer over 4 psum banks.
            ps2 = [
                psum_pool.tile(
                    [128, 512], dt.float32, tag="ps", name=f"ps_2_{bt}"
                )
                for bt in range(4)
            ]
            for k in range(KT // 2):
                for bt in range(4):
                    nc.tensor.matmul(
                        ps2[bt][:],
                        xbt_sb[:, 2 * k : 2 * k + 2, 128 * bt : 128 * (bt + 1)],
                        wd_sb[:, 2, 2 * k : 2 * k + 2, :],
                        start=(k == 0),
                        stop=(k == KT // 2 - 1),
                        perf_mode=DR,
                    )
            for bt in range(4):
                drain(ps2[bt], bt, 1024, 512)

            # Final o-quarter: k-serial per batch tile; last batch tile
            # as 2 N=256 chains for a small final drain.
            for bt in range(3):
                ps = psum_pool.tile(
                    [128, 512], dt.float32, tag="ps", name=f"ps_3_{bt}"
                )
                for k in range(KT // 2):
                    nc.tensor.matmul(
                        ps[:],
                        xbt_sb[:, 2 * k : 2 * k + 2, 128 * bt : 128 * (bt + 1)],
                        wd_sb[:, 3, 2 * k : 2 * k + 2, :],
                        start=(k == 0),
                        stop=(k == KT // 2 - 1),
                        perf_mode=DR,
                    )
                drain(ps, bt, 1536, 512)
            for q, (ocol, width) in enumerate(((1536, 384), (1920, 128))):
                ps = psum_pool.tile(
                    [128, width], dt.float32, tag="ps", name=f"ps_3_3_{q}"
                )
                for k in range(KT // 2):
                    nc.tensor.matmul(
                        ps[:],
                        xbt_sb[:, 2 * k : 2 * k + 2, 384:512],
                        wd_sb[:, 3, 2 * k : 2 * k + 2, ocol - 1536 : ocol - 1536 + width],
                        start=(k == 0),
                        stop=(k == KT // 2 - 1),
                        perf_mode=DR,
                    )
                drain(ps, 3, ocol, width)
    nc.compile()
    return nc


def get_nc():
    if "nc" not in _cached:
        _cached["nc"] = _build_nc()
    return _cached["nc"]


def pack_inputs(x, w):
    """Host-side bit unpack + layout. Returns (xbt, wd_cores, bias)."""
    xb = np.unpackbits(
        x.view(np.uint8).reshape(BATCH, IN_INTS, 8), axis=-1, bitorder="little"
    ).reshape(BATCH, K)
    xbt = np.ascontiguousarray(
        xb.T.reshape(4, 4, 128, BATCH).transpose(0, 2, 1, 3)
    ).astype(_FP8)

    wbits = np.unpackbits(
        w.view(np.uint8).reshape(POP, IN_INTS, 2, OUT_F, 8),
        axis=-1,
        bitorder="little",
    )
    w0 = wbits[:, :, 0].transpose(0, 1, 3, 2).reshape(POP, K, OUT_F)
    w1 = wbits[:, :, 1].transpose(0, 1, 3, 2).reshape(POP, K, OUT_F)
    bias = w1.sum(axis=1, dtype=np.int32)
    wd = w0.astype(np.int8) - w1.astype(np.int8)
    wd_cores = [
        np.ascontiguousarray(
            wd[p].reshape(4, 4, 128, 4, 512).transpose(0, 3, 2, 1, 4)
        ).astype(_FP8)
        for p in range(POP)
    ]
    return xbt, wd_cores, bias


def kernel(x, w):
    from concourse.bass_utils import run_bass_kernel_spmd

    nc = get_nc()
    xbt, wd_cores, bias = pack_inputs(np.asarray(x), np.asarray(w))
    in_maps = [{"xbt": xbt, "wd": wd_cores[p]} for p in range(N_CORES)]
    try:
        res = run_bass_kernel_spmd(nc, in_maps, list(range(N_CORES)))
    except Exception:
        res = run_bass_kernel_spmd(nc, in_maps, list(range(N_CORES)))
    out = np.empty((POP, BATCH, OUT_F), dtype=np.int32)
    for p in range(N_CORES):
        out[p] = res.results[p]["out"].astype(np.int32) + bias[p][None, :]
    return out
